# revision 31
# baseline (speedup 1.0000x reference)
"""Trainium2 Bass kernel for NemotronFlash Mamba decoder layer.

Sharding: 8 cores = 2 batches x 4 sequence shards of 512 tokens.
All compute is shard-local except the SSD inter-chunk state, which is
exchanged via one AllGather of (L_k, D_k) within each 4-core batch group.

v2 restructure vs baseline:
 - in-proj m-tile order: dt first, then xBC (conv overlapped on PE via
   diagonal matmuls), z last -> collective launches ~4x earlier.
 - Y produced directly transposed (swapped matmul operands), removing
   64 DMA transposes.
 - acs partition-broadcasts (R_all) prefetched once and shared by the
   segment-exp (eL) and state-exp (ebc) paths.
 - gating rmsnorm row broadcast via a K=1 PE matmul instead of a DRAM
   round trip; silu fused PSUM->SBUF; act-table switches minimized.
"""
import sys
import numpy as np

sys.path.insert(0, "/opt/trn_rl_repo")

from contextlib import ExitStack  # noqa: E402
import ml_dtypes  # noqa: E402
import concourse.bass as bass  # noqa: E402
import concourse.mybir as mybir  # noqa: E402
import concourse.tile as tile  # noqa: E402
from concourse import bacc  # noqa: E402
from concourse.bass_utils import run_bass_kernel_spmd  # noqa: E402

F32 = mybir.dt.float32
BF16 = mybir.dt.bfloat16
AF = mybir.ActivationFunctionType
OP = mybir.AluOpType

H = 1024
E = 2048
NH = 32
P = 64
NST = 128          # d_state
KC = 4             # d_conv
Q = 128            # chunk len
FF = 4096
CONV = E + 2 * NST          # 2304
D_IN = 2 * E + 2 * NST + NH  # 4384
EPS = 1e-6
NEPS = 1e-5
LSEQ = 512         # tokens per shard
NCHUNK = LSEQ // Q  # 4
NROW = 5           # 5 row tiles of 128 = 640 padded rows
HALO = 3
NCORES = 8

NZT = E // Q       # 16 z tiles
NXT = CONV // Q    # 18 xBC tiles
NMT = 35           # in-proj m tiles (1 dt + 18 xBC + 16 z)
NGRP = 9           # in-proj column groups of 512
NKH = H // Q       # 8 k tiles over H
NKE = E // Q       # 16 k tiles over E
NFT = FF // Q      # 32 FF tiles
HG = 8             # heads per SSD group
NG = NH // HG      # 4 groups


def row_bcast(ap_row, parts=128):
    """AP broadcasting a [1, n] row across `parts` partitions (step-0)."""
    return bass.AP(tensor=ap_row.tensor, offset=ap_row.offset,
                   ap=[[0, parts]] + [list(x) for x in ap_row.ap[1:]])


def colbc(src_ap, n, rep):
    # [128, n, rep] broadcast of per-head columns along a new axis
    return bass.AP(tensor=src_ap.tensor, offset=src_ap.offset,
                   ap=[list(src_ap.ap[0])] + [[1, n], [0, rep]])


def rowbc(src_ap, rep, n):
    # [128, rep, n] broadcast of a [128, n] tile along middle axis
    return bass.AP(tensor=src_ap.tensor, offset=src_ap.offset,
                   ap=[list(src_ap.ap[0])] + [[0, rep], [1, n]])


def build_program(dvals):
    nc = bacc.Bacc("TRN2", target_bir_lowering=False, debug=False,
                   num_devices=NCORES)

    hs_in = nc.dram_tensor("hs", [NROW * 128, H], F32, kind="ExternalInput")
    wiT = nc.dram_tensor("wiT", [NGRP * 128, NKH * 512], BF16,
                         kind="ExternalInput")
    woT = nc.dram_tensor("woT", [E, H], BF16, kind="ExternalInput")
    wgT = nc.dram_tensor("wgT", [NFT * 128, NKH * 128], BF16,
                         kind="ExternalInput")
    wuT = nc.dram_tensor("wuT", [NFT * 128, NKH * 128], BF16,
                         kind="ExternalInput")
    wdT = nc.dram_tensor("wdT", [FF, H], BF16, kind="ExternalInput")
    wdiag = nc.dram_tensor("wdiag", [128, NXT * KC * 128], BF16,
                           kind="ExternalInput")
    bconv = nc.dram_tensor("bconv", [128, NXT], F32, kind="ExternalInput")
    avec = nc.dram_tensor("avec", [NH, 1], F32, kind="ExternalInput")
    dtb = nc.dram_tensor("dtb", [NH, 1], F32, kind="ExternalInput")
    mask8 = nc.dram_tensor("mask8", [128, 8], F32, kind="ExternalInput")
    negmask = nc.dram_tensor("negmask", [128, 128], F32, kind="ExternalInput")
    idf32 = nc.dram_tensor("idf32", [128, 128], F32, kind="ExternalInput")
    ddiag_in = nc.dram_tensor("ddiag", [128, NKE * 128], BF16,
                              kind="ExternalInput")
    out_d = nc.dram_tensor("out", [LSEQ, H], F32, kind="ExternalOutput")

    with tile.TileContext(nc) as tc, ExitStack() as stack:
        consts = stack.enter_context(tc.tile_pool(name="consts", bufs=1))
        bconv_sb = consts.tile([128, NXT], F32)
        nc.scalar.dma_start(out=bconv_sb[:], in_=bconv[:])
        avec_sb = consts.tile([NH, 1], F32)
        nc.gpsimd.dma_start(out=avec_sb[:], in_=avec[:])
        dtb_sb = consts.tile([NH, 1], F32)
        nc.gpsimd.dma_start(out=dtb_sb[:], in_=dtb[:])
        mask_sb = consts.tile([128, 8], F32)
        nc.gpsimd.dma_start(out=mask_sb[:], in_=mask8[:])
        nm_sb = consts.tile([128, 128], F32)
        nc.scalar.dma_start(out=nm_sb[:], in_=negmask[:])
        id_sb = consts.tile([128, 128], F32)
        nc.scalar.dma_start(out=id_sb[:], in_=idf32[:])
        ddiag_sb = consts.tile([128, NKE * 128], BF16)
        nc.gpsimd.dma_start(out=ddiag_sb[:], in_=ddiag_in[:])
        ones_bf = consts.tile([128, 1], BF16)
        nc.vector.memset(ones_bf[:], 1.0)
        ones1f = consts.tile([1, 128], F32)
        nc.vector.memset(ones1f[:], 1.0)
        zero32 = consts.tile([NH, Q], F32)
        nc.vector.memset(zero32[:], 0.0)
        epsc = consts.tile([128, 1], F32)
        nc.vector.memset(epsc[:], EPS)
        nepsc = consts.tile([128, 1], F32)
        nc.vector.memset(nepsc[:], NEPS)

        ccdram = stack.enter_context(
            tc.tile_pool(name="ccdram", bufs=1, space="DRAM"))
        cc_in = ccdram.tile([128, E + 1], BF16)
        cc_out = ccdram.tile([4, 128, E + 1], BF16)
        acs_d = ccdram.tile([NH, LSEQ], F32)
        drow_d = ccdram.tile([1, 4 * NH], F32)

        es_sz = ExitStack()
        pSZ = es_sz.enter_context(tc.tile_pool(name="pSZ", bufs=1))
        szT = pSZ.tile([128, NZT, LSEQ], BF16)

        es_yt = ExitStack()
        pYT = es_yt.enter_context(tc.tile_pool(name="pYT", bufs=1))
        yT = pYT.tile([128, NKE, LSEQ], BF16)

        es_go = ExitStack()                        # hsr lives until G
        pGo = es_go.enter_context(tc.tile_pool(name="pGo", bufs=1))
        hsr = pGo.tile([128, 4, 2, 512], F32)

        es_mid = ExitStack()                       # B .. F2
        pMID = es_mid.enter_context(tc.tile_pool(name="pMID", bufs=1))
        dtraw = pMID.tile([NH, LSEQ], F32)
        dt_sb = pMID.tile([NH, LSEQ], F32)
        acs = pMID.tile([NH, LSEQ], F32)
        xcbc = pMID.tile([128, 2, LSEQ], BF16)
        G_sb = pMID.tile([128, NCHUNK, Q], BF16)
        x_tm = pMID.tile([128, NCHUNK, E], BF16)
        B_tm = pMID.tile([128, NCHUNK, NST], BF16)
        dtacsT = pMID.tile([128, NCHUNK, 3 * NH], F32)
        wdtb = pMID.tile([128, NCHUNK, NH], BF16)
        dtb16 = pMID.tile([128, NCHUNK, NH], BF16)
        dcstb = pMID.tile([128, NCHUNK, NH], BF16)
        pdec = pMID.tile([128, NCHUNK - 1, NH], BF16)
        cstates = pMID.tile([128, NCHUNK, E], BF16)
        dkcol = pMID.tile([NH, 1], BF16)

        es_xc = ExitStack()                        # conv .. F2
        pXC = es_xc.enter_context(tc.tile_pool(name="pXC", bufs=1))
        xc = pXC.tile([128, NZT, LSEQ], BF16)

        es_hT = ExitStack()
        pHT = es_hT.enter_context(tc.tile_pool(name="pHT", bufs=1))
        hT = pHT.tile([128, NKH, NROW * 128], BF16)

        tq = [nc.sync, nc.gpsimd, nc.scalar]
        ttq = [nc.sync, nc.sync]

        # ---------------- Phase A: rmsnorm1 + h^T ----------------
        with tc.tile_pool(name="pA", bufs=3) as pA, \
             tc.tile_pool(name="stat", bufs=4) as stat:
            for r in range(NROW):
                hst = pA.tile([128, H], F32, tag="hst")
                nc.sync.dma_start(out=hst[:], in_=hs_in[r * 128:(r + 1) * 128, :])
                sq = pA.tile([128, H], F32, tag="sq", bufs=2)
                ssum = stat.tile([128, 1], F32, tag="ssum")
                nc.scalar.activation(out=sq[:], in_=hst[:], func=AF.Square,
                                     accum_out=ssum[:])
                rs = stat.tile([128, 1], F32, tag="rs")
                nc.scalar.activation(out=rs[:], in_=ssum[:], func=AF.Sqrt,
                                     scale=1.0 / H, bias=epsc[:])
                nc.vector.reciprocal(rs[:], rs[:])
                hbf = pA.tile([128, H], BF16, tag="hbf")
                nc.vector.tensor_scalar_mul(hbf[:], hst[:], rs[:])
                ttq[r % 2].dma_start_transpose(
                    hT[:, :, r * 128:(r + 1) * 128], hbf[:])

        # ================= Phase B/C/D fused pipeline =================
        # m-tile order: 0=dt, 1..18=xBC j, 19..34=z
        es_bw = ExitStack()
        wip = es_bw.enter_context(tc.tile_pool(name="wip", bufs=3))
        psB = es_bw.enter_context(tc.tile_pool(name="psB", bufs=3,
                                               space="PSUM"))
        psBh = es_bw.enter_context(tc.tile_pool(name="psBh", bufs=1,
                                                space="PSUM"))
        psC = es_bw.enter_context(tc.tile_pool(name="psC", bufs=2,
                                               space="PSUM"))
        xbcf = es_bw.enter_context(tc.tile_pool(name="xbcf", bufs=1))
        xbc = xbcf.tile([128, NXT, HALO + LSEQ], BF16)
        wdiag_sb = xbcf.tile([128, NXT * KC * 128], BF16)
        nc.scalar.dma_start(out=wdiag_sb[:], in_=wdiag[:])

        def emit_conv(j):
            # 4-tap causal depthwise conv as diagonal matmuls + fused silu
            pc = psC.tile([128, LSEQ], F32, tag="pc")
            for k in range(KC):
                nc.tensor.matmul(
                    pc[:], wdiag_sb[:, (j * KC + k) * 128:(j * KC + k + 1) * 128],
                    xbc[:, j, k:k + LSEQ],
                    start=(k == 0), stop=(k == KC - 1))
            dst = xc[:, j, :] if j < NZT else xcbc[:, j - NZT, :]
            nc.scalar.activation(out=dst, in_=pc[:], func=AF.Silu,
                                 bias=bconv_sb[:, j:j + 1])
            if j < NZT:
                eng = nc.sync if j % 2 == 0 else nc.scalar
                eng.dma_start_transpose(
                    x_tm[:, :, j * 128:(j + 1) * 128], xc[:, j, :])
            elif j == NZT:
                nc.scalar.dma_start_transpose(B_tm[:], xcbc[:, 0, :])

        def emit_C():
            # dt path: softplus, dA cumsum, transposes, decay factors
            with tc.high_priority(), \
                 tc.tile_pool(name="pC", bufs=2) as pC, \
                 tc.tile_pool(name="psCt", bufs=2, space="PSUM") as psCt:
                e1 = pC.tile([NH, LSEQ], F32, tag="e1")
                nc.scalar.activation(out=e1[:], in_=dtraw[:], func=AF.Exp,
                                     bias=dtb_sb[:])
                nc.vector.tensor_scalar_add(e1[:], e1[:], 1.0)
                nc.scalar.activation(out=dt_sb[:], in_=e1[:], func=AF.Ln)
                dA = pC.tile([NH, LSEQ], F32, tag="dA")
                nc.vector.tensor_scalar_mul(dA[:], dt_sb[:], avec_sb[:])
                for c in range(NCHUNK):
                    nc.vector.tensor_tensor_scan(
                        acs[:, c * Q:(c + 1) * Q], dA[:, c * Q:(c + 1) * Q],
                        zero32[:], 0.0, OP.add, OP.add)
                nc.sync.dma_start(out=acs_d[:], in_=acs[:])
                stk = pC.tile([3 * NH, LSEQ], F32, tag="stk")
                nc.vector.tensor_copy(stk[0:NH, :], dt_sb[:])
                nc.vector.tensor_copy(stk[NH:2 * NH, :], acs[:])
                # rev[h, c, q] = acs[h, c, Q-1] - acs[h, c, q]
                av = acs[:]
                lastcol = bass.AP(tensor=av.tensor, offset=av.offset + Q - 1,
                                  ap=[list(av.ap[0]), [Q, NCHUNK], [0, Q]])
                nc.vector.tensor_tensor(
                    out=stk[2 * NH:3 * NH, :].rearrange(
                        "p (c q) -> p c q", c=NCHUNK),
                    in0=lastcol,
                    in1=acs[:].rearrange("p (c q) -> p c q", c=NCHUNK),
                    op=OP.subtract)
                for c in range(NCHUNK):
                    pst = psCt.tile([128, 3 * NH], F32, tag="pst")
                    nc.tensor.transpose(pst[:], stk[:, c * Q:(c + 1) * Q],
                                        id_sb[0:3 * NH, 0:3 * NH])
                    nc.scalar.copy(dtacsT[:, c, :], pst[:])
                nc.vector.tensor_copy(dtb16[:], dtacsT[:, :, 0:NH])
                # whole-shard per-head decay (for the collective payload)
                acs4 = acs[:].rearrange("p (c q) -> p c q", c=NCHUNK)[:, :, Q - 1]
                asum = pC.tile([NH, 1], F32, tag="asum")
                nc.vector.tensor_reduce(asum[:], acs4,
                                        axis=mybir.AxisListType.X, op=OP.add)
                nc.scalar.activation(out=dkcol[:], in_=asum[:], func=AF.Exp)

        def emit_C2():
            # chunk-decay factors from transposed rev-cumsum (no broadcast)
            with tc.high_priority(), \
                 tc.tile_pool(name="pC2", bufs=1) as pC2:
                decT = pC2.tile([128, NCHUNK, NH], F32, tag="decT")
                nc.scalar.activation(out=decT[:],
                                     in_=dtacsT[:, :, 2 * NH:3 * NH],
                                     func=AF.Exp)
                eacsT = pC2.tile([128, NCHUNK, NH], F32, tag="eacsT")
                nc.scalar.activation(out=eacsT[:],
                                     in_=dtacsT[:, :, NH:2 * NH],
                                     func=AF.Exp)
                nc.vector.tensor_mul(wdtb[:], decT[:], dtacsT[:, :, 0:NH])
                nc.vector.tensor_mul(dcstb[:], decT[:], eacsT[:])
                nc.vector.tensor_copy(pdec[:, 0, :], dcstb[:, 0, :])
                nc.vector.tensor_mul(pdec[:, 1, :], pdec[:, 0, :],
                                     dcstb[:, 1, :])
                nc.vector.tensor_mul(pdec[:, 2, :], pdec[:, 1, :],
                                     dcstb[:, 2, :])

        def emit_eL(c, g):
            # segment matrix -> mt (bf16) for chunk c, head group g
            h0 = g * HG
            ra = pRA.tile([128, HG, Q], F32, tag="ra")
            ad2 = acs_d[:]
            tq[(c * NG + g) % 2].dma_start(
                out=ra[:],
                in_=bass.AP(tensor=ad2.tensor,
                            offset=ad2.offset + h0 * LSEQ + c * Q,
                            ap=[[0, 128], [LSEQ, HG], [1, Q]]))
            ebct = pEL.tile([128, HG, Q], BF16, tag="ebct")
            nc.scalar.activation(out=ebct[:], in_=ra[:], func=AF.Exp)
            nc.vector.tensor_mul(
                CT[:, c, h0:h0 + HG, :], ebct[:],
                rowbc(xcbc[:, 1, c * Q:(c + 1) * Q], HG, Q))
            segm = pEL.tile([128, HG, Q], BF16, tag="segm")
            for hh in range(HG):
                nc.vector.tensor_scalar(
                    segm[:, hh, :], ra[:, hh, :],
                    dtacsT[:, c, NH + h0 + hh:NH + h0 + hh + 1], 0.0,
                    OP.subtract, OP.min)
            eLt = pEL.tile([128, HG, Q], BF16, tag="eL")
            nc.scalar.activation(out=eLt[:], in_=segm[:], func=AF.Exp)
            mt = pMT.tile([128, HG, Q], BF16, tag="mt")
            nc.vector.tensor_mul(mt[:], eLt[:], rowbc(G_sb[:, c, :], HG, Q))
            return mt

        def emit_E():
            # per-chunk states + L combine + collective launch
            with tc.high_priority(), \
                 tc.tile_pool(name="psE", bufs=2, space="PSUM") as psE, \
                 tc.tile_pool(name="pE", bufs=2) as pE:
                for c in range(NCHUNK):
                    xdd = pE.tile([128, NH, P], BF16, tag="xdd")
                    nc.vector.tensor_mul(
                        xdd[:], x_tm[:, c, :].rearrange("p (h q) -> p h q", h=NH),
                        colbc(wdtb[:, c, :], NH, P))
                    for g in range(NG):
                        ps_st = psE.tile([128, 512], F32, tag="ps_st")
                        nc.tensor.matmul(
                            ps_st[:], B_tm[:, c, :],
                            xdd[:, g * HG:(g + 1) * HG, :],
                            start=True, stop=True)
                        if g % 2 == 0:
                            nc.scalar.copy(
                                cstates[:, c, g * 512:(g + 1) * 512], ps_st[:])
                        else:
                            nc.vector.tensor_copy(
                                cstates[:, c, g * 512:(g + 1) * 512], ps_st[:])
                Lacc = cstates[:, 0, :]
                for c in range(1, NCHUNK):
                    t1 = pE.tile([128, NH, P], BF16, tag="lt", name=f"lt{c}",
                                 bufs=1)
                    nc.vector.tensor_mul(
                        t1[:], Lacc.rearrange("p (h q) -> p h q", h=NH),
                        colbc(dcstb[:, c, :], NH, P))
                    nc.vector.tensor_add(
                        cstates[:, c, :].rearrange("p (h q) -> p h q", h=NH),
                        t1[:],
                        cstates[:, c, :].rearrange("p (h q) -> p h q", h=NH))
                    Lacc = cstates[:, c, :]
                nc.gpsimd.dma_start(out=cc_in[:, 0:E], in_=Lacc)
                nc.gpsimd.dma_start(out=cc_in[0:NH, E:E + 1], in_=dkcol[:])
                nc.gpsimd.collective_compute(
                    "AllGather", OP.bypass,
                    replica_groups=[[0, 1, 2, 3], [4, 5, 6, 7]],
                    ins=[cc_in.opt()], outs=[cc_out.opt()])

        # ---- emit B groups with interleaved conv / C / E ----
        for gi in range(NGRP):
            g0, g1 = gi * 4, min(gi * 4 + 4, NMT)
            wi_g = wip.tile([128, NKH, 512], BF16, tag="wi")
            nc.gpsimd.dma_start(out=wi_g[:],
                                in_=wiT[gi * 128:(gi + 1) * 128, :])
            for mm in range(g0, g1):
                moff = (mm - g0) * 128
                ps = psB.tile([128, LSEQ], F32, tag="ps")
                for k in range(NKH):
                    nc.tensor.matmul(
                        ps[:], wi_g[:, k, moff:moff + 128],
                        hT[:, k, HALO:HALO + LSEQ],
                        start=(k == 0), stop=(k == NKH - 1))
                if mm == 0:
                    nc.scalar.copy(dtraw[:], ps[0:NH, :])
                    emit_C()
                elif mm <= NXT:
                    j = mm - 1
                    psh = psBh.tile([128, HALO], F32, tag="psh")
                    for k in range(NKH):
                        nc.tensor.matmul(
                            psh[:], wi_g[:, k, moff:moff + 128],
                            hT[:, k, 0:HALO],
                            start=(k == 0), stop=(k == NKH - 1))
                    if j % 2 == 0:
                        nc.scalar.copy(xbc[:, j, HALO:], ps[:])
                    else:
                        nc.vector.tensor_copy(xbc[:, j, HALO:], ps[:])
                    nc.scalar.copy(xbc[:, j, 0:HALO], psh[:])
                    if j >= 1:
                        emit_conv(j - 1)
                    if j == NXT - 1:
                        emit_conv(NXT - 1)
                        # G = (B^T C) masked, per chunk
                        with tc.tile_pool(name="psG", bufs=2,
                                          space="PSUM") as psGm:
                            for c in range(NCHUNK):
                                gps = psGm.tile([128, Q], F32, tag="gps")
                                nc.tensor.matmul(
                                    gps[:], xcbc[:, 0, c * Q:(c + 1) * Q],
                                    xcbc[:, 1, c * Q:(c + 1) * Q],
                                    start=True, stop=True)
                                nc.vector.tensor_mul(G_sb[:, c, :], gps[:],
                                                     nm_sb[:])
                else:
                    mz = mm - 1 - NXT
                    nc.scalar.activation(out=szT[:, mz, :], in_=ps[:],
                                         func=AF.Silu)
            if gi == 2:
                emit_C2()
            if gi == 6:
                emit_E()

        es_bw.close()
        es_hT.close()

        # ---------------- Phase F1: Y_diag^T ----------------
        hv = hs_in[:]
        nc.gpsimd.dma_start(
            out=hsr[:],
            in_=bass.AP(tensor=hv.tensor, offset=hv.offset + HALO * H,
                        ap=[[H, 128], [128 * H, 4], [512, 2], [1, 512]]))
        es_ct = ExitStack()    # Ct = exp(acs)*C, consumed in F2
        pCT = es_ct.enter_context(tc.tile_pool(name="pCT", bufs=1))
        CT = pCT.tile([128, NCHUNK, NH, Q], BF16)
        es_eb = ExitStack()    # eL-window pools (close after F1)
        pEL = es_eb.enter_context(tc.tile_pool(name="pEL", bufs=4))
        pMT = es_eb.enter_context(tc.tile_pool(name="pMT", bufs=6))
        pRA = es_eb.enter_context(tc.tile_pool(name="pRA", bufs=4))
        pXD = es_eb.enter_context(tc.tile_pool(name="pXD", bufs=2))
        es_ps = ExitStack()
        psY = es_ps.enter_context(tc.tile_pool(name="psY", bufs=3,
                                               space="PSUM"))
        psY2 = es_ps.enter_context(tc.tile_pool(name="psY2", bufs=4,
                                                space="PSUM"))
        for c in range(NCHUNK):
            xd = pXD.tile([128, NH, P], BF16, tag="xd")
            nc.vector.tensor_mul(
                xd[:], x_tm[:, c, :].rearrange("p (h q) -> p h q", h=NH),
                colbc(dtb16[:, c, :], NH, P))
            for g in range(NG):
                mt = emit_eL(c, g)
                psyT = psY.tile([128, NG, Q], F32, tag="psyT")
                for hh in range(HG):
                    h = g * HG + hh
                    nc.tensor.matmul(
                        psyT[(hh % 2) * 64:(hh % 2) * 64 + 64, hh // 2, :],
                        xd[:, h, :], mt[:, hh, :], start=True, stop=False,
                        skip_group_check=True)
                for jj in range(NG):
                    j = g * NG + jj
                    nc.tensor.matmul(
                        psyT[:, jj, :],
                        ddiag_sb[:, j * 128:(j + 1) * 128],
                        xc[:, j, c * Q:(c + 1) * Q],
                        start=False, stop=(jj == NG - 1),
                        skip_group_check=True)
                nc.vector.tensor_copy(
                    yT[:, g * NG:(g + 1) * NG, c * Q:(c + 1) * Q], psyT[:])
        es_eb.close()

        # ---------------- S_init combine ----------------
        es_sb = ExitStack()
        sbfp = es_sb.enter_context(tc.tile_pool(name="sbfp", bufs=4))
        Sbf = None
        with tc.tile_pool(name="pS", bufs=1) as pS:
            Lg = pS.tile([128, 4, E], BF16, tag="Lg")
            Dg = pS.tile([NH, 4], BF16, tag="Dg")
            for j in range(4):
                nc.sync.dma_start(out=Lg[:, j, :], in_=cc_out[j, :, 0:E])
                nc.sync.dma_start(out=Dg[:, j:j + 1],
                                  in_=cc_out[j, 0:NH, E:E + 1])
            deff = pS.tile([NH, 4], F32, tag="deff")
            for j in range(4):
                nc.vector.scalar_tensor_tensor(
                    out=deff[:, j:j + 1], in0=Dg[:, j:j + 1],
                    scalar=mask_sb[0:NH, j:j + 1],
                    in1=mask_sb[0:NH, 4 + j:5 + j],
                    op0=OP.mult, op1=OP.add)
            for j in range(4):
                nc.sync.dma_start(out=drow_d[0:1, j * NH:(j + 1) * NH],
                                  in_=deff[:, j:j + 1])
            drow_sb = pS.tile([1, 4 * NH], F32, tag="drow")
            nc.sync.dma_start(out=drow_sb[:], in_=drow_d[:])
            dbcp = psY.tile([128, 4 * NH], F32, tag="dbcp", bufs=1)
            nc.tensor.matmul(dbcp[:], ones1f[:], drow_sb[:],
                             start=True, stop=True)
            dbc = dbcp
            Sacc = pS.tile([128, E], BF16, tag="sacc", name="sacc0")
            nc.vector.tensor_scalar_mul(Sacc[:], Lg[:, 0, :], mask_sb[:, 0:1])
            for j in range(1, 4):
                t2 = pS.tile([128, NH, P], BF16, tag="st2", name=f"st2{j}")
                nc.vector.tensor_mul(
                    t2[:], Sacc[:].rearrange("p (h q) -> p h q", h=NH),
                    colbc(dbc[:, j * NH:(j + 1) * NH], NH, P))
                lf = pS.tile([128, E], BF16, tag="lf", name=f"lf{j}")
                nc.vector.tensor_scalar_mul(lf[:], Lg[:, j, :],
                                            mask_sb[:, j:j + 1])
                Sacc = pS.tile([128, E], BF16, tag="sacc2", name=f"sacc{j}")
                nc.vector.tensor_add(
                    Sacc[:].rearrange("p (h q) -> p h q", h=NH), t2[:],
                    lf[:].rearrange("p (h q) -> p h q", h=NH))
            Sbf = sbfp.tile([128, E], BF16, tag="sbf", name="sbf0")
            nc.vector.tensor_copy(Sbf[:], Sacc[:])

        # ---------------- Phase F2: Y_off^T + state updates ----------------
        with tc.tile_pool(name="pF2", bufs=3) as pF2:
            Ss = [Sbf]
            for c in range(1, NCHUNK):
                st = pF2.tile([128, NH, P], BF16, tag="stmp")
                nc.vector.tensor_mul(
                    st[:], Sbf[:].rearrange("p (h q) -> p h q", h=NH),
                    colbc(pdec[:, c - 1, :], NH, P))
                Sc = sbfp.tile([128, E], BF16, tag="sbf", name=f"sbf{c}")
                nc.vector.tensor_add(
                    Sc[:].rearrange("p (h q) -> p h q", h=NH), st[:],
                    cstates[:, c - 1, :].rearrange("p (h q) -> p h q", h=NH))
                Ss.append(Sc)
            for c in range(NCHUNK):
                Sc = Ss[c]
                for g in range(NG):
                    h0 = g * HG
                    psy2 = psY2.tile([128, NG, Q], F32, tag="psy2")
                    for hh in range(HG):
                        h = h0 + hh
                        nc.tensor.matmul(
                            psy2[(hh % 2) * 64:(hh % 2) * 64 + 64, hh // 2, :],
                            Sc[:, h * P:(h + 1) * P], CT[:, c, h, :],
                            start=True, stop=True)
                    nc.vector.tensor_add(
                        yT[:, g * NG:(g + 1) * NG, c * Q:(c + 1) * Q],
                        yT[:, g * NG:(g + 1) * NG, c * Q:(c + 1) * Q],
                        psy2[:])
        es_ps.close()
        es_sb.close()
        es_ct.close()
        es_xc.close()
        es_mid.close()

        # ---------------- Phase G: gating + norm + out-proj ----------------
        es_g = ExitStack()
        pGt = es_g.enter_context(tc.tile_pool(name="pGt", bufs=1))
        gt = pGt.tile([128, NKE, LSEQ], BF16)
        pH2 = es_g.enter_context(tc.tile_pool(name="pH2", bufs=1))
        h2 = pH2.tile([128, NCHUNK, H], F32)
        h2nT = pH2.tile([128, NKH, LSEQ], BF16)
        pGoW = es_g.enter_context(tc.tile_pool(name="pGoW", bufs=1))
        wo_sb = pGoW.tile([128, NKE, H], BF16)
        wv = woT[:]
        nc.scalar.dma_start(
            out=wo_sb[:],
            in_=bass.AP(tensor=wv.tensor, offset=wv.offset,
                        ap=[[H, 128], [128 * H, NKE], [1, H]]))
        with tc.tile_pool(name="pGa", bufs=3) as pGa, \
             tc.tile_pool(name="psN", bufs=1, space="PSUM") as psN, \
             tc.tile_pool(name="psR", bufs=1, space="PSUM") as psR, \
             tc.tile_pool(name="psO", bufs=3, space="PSUM") as psO, \
             tc.tile_pool(name="stat2", bufs=4) as stat2:
            sqps = psN.tile([128, LSEQ], F32)
            for mz in range(NKE):
                nc.vector.tensor_mul(gt[:, mz, :], yT[:, mz, :], szT[:, mz, :])
                g2 = pGa.tile([128, LSEQ], BF16, tag="g2")
                nc.scalar.activation(out=g2[:], in_=gt[:, mz, :],
                                     func=AF.Square)
                nc.tensor.matmul(sqps[0:1, :], ones_bf[:], g2[:],
                                 start=(mz == 0), stop=(mz == NKE - 1))
            rsrow = stat2.tile([1, LSEQ], F32, tag="rsrow")
            nc.scalar.activation(out=rsrow[:], in_=sqps[0:1, :], func=AF.Sqrt,
                                 scale=1.0 / E, bias=nepsc[0:1, :])
            nc.vector.reciprocal(rsrow[:], rsrow[:])
            rsbc = psR.tile([128, LSEQ], F32)
            nc.tensor.matmul(rsbc[:], ones1f[:], rsrow[:],
                             start=True, stop=True)
            for mz in range(NKE):
                nc.vector.tensor_mul(gt[:, mz, :], gt[:, mz, :], rsbc[:])
            for tt in range(NCHUNK):
                for half in range(2):
                    ps = psO.tile([128, 512], F32, tag="po")
                    for k in range(NKE):
                        nc.tensor.matmul(
                            ps[:], gt[:, k, tt * 128:(tt + 1) * 128],
                            wo_sb[:, k, half * 512:(half + 1) * 512],
                            start=(k == 0), stop=(k == NKE - 1))
                    nc.vector.tensor_add(
                        h2[:, tt, half * 512:(half + 1) * 512],
                        ps[:], hsr[:, tt, half, :])
            # rms2 + transpose
            for tt in range(NCHUNK):
                sq2 = pGa.tile([128, H], F32, tag="sq2")
                ss2 = stat2.tile([128, 1], F32, tag="ss2")
                nc.scalar.activation(out=sq2[:], in_=h2[:, tt, :],
                                     func=AF.Square, accum_out=ss2[:])
                rs2 = stat2.tile([128, 1], F32, tag="rs2")
                nc.scalar.activation(out=rs2[:], in_=ss2[:], func=AF.Sqrt,
                                     scale=1.0 / H, bias=epsc[:])
                nc.vector.reciprocal(rs2[:], rs2[:])
                h2n = pGa.tile([128, H], BF16, tag="h2n")
                nc.vector.tensor_scalar_mul(h2n[:], h2[:, tt, :], rs2[:])
                ttq[tt % 2].dma_start_transpose(
                    h2nT[:, :, tt * 128:(tt + 1) * 128], h2n[:])

        pGU = es_g.enter_context(tc.tile_pool(name="pGU", bufs=1))
        gu = pGU.tile([128, NFT, LSEQ], BF16)

        # ---------------- Phase H: MLP ----------------
        with tc.tile_pool(name="wmP", bufs=4) as wmP, \
             tc.tile_pool(name="psM", bufs=4, space="PSUM") as psM, \
             tc.tile_pool(name="pM", bufs=3) as pM:
            for mf in range(NFT):
                wg_m = wmP.tile([128, NKH, 128], BF16, tag="wg")
                nc.sync.dma_start(out=wg_m[:],
                                  in_=wgT[mf * 128:(mf + 1) * 128, :])
                wu_m = wmP.tile([128, NKH, 128], BF16, tag="wu")
                nc.gpsimd.dma_start(out=wu_m[:],
                                    in_=wuT[mf * 128:(mf + 1) * 128, :])
                gps = psM.tile([128, LSEQ], F32, tag="gps")
                for k in range(NKH):
                    nc.tensor.matmul(gps[:], wg_m[:, k, :], h2nT[:, k, :],
                                     start=(k == 0), stop=(k == NKH - 1))
                sg = pM.tile([128, LSEQ], BF16, tag="sg")
                nc.scalar.activation(out=sg[:], in_=gps[:], func=AF.Silu)
                ups = psM.tile([128, LSEQ], F32, tag="ups")
                for k in range(NKH):
                    nc.tensor.matmul(ups[:], wu_m[:, k, :], h2nT[:, k, :],
                                     start=(k == 0), stop=(k == NKH - 1))
                nc.vector.tensor_mul(gu[:, mf, :], sg[:], ups[:])
        with tc.tile_pool(name="wdP", bufs=4) as wdP, \
             tc.tile_pool(name="psD", bufs=1, space="PSUM") as psD, \
             tc.tile_pool(name="pO", bufs=4) as pO:
            dps = []
            for i in range(8):
                dpt = psD.tile([128, 512], F32, tag=f"dp{i}", name=f"dp{i}")
                dps.append(dpt)
            for k in range(NFT):
                wd_k = wdP.tile([128, H], BF16, tag="wd")
                nc.sync.dma_start(out=wd_k[:],
                                  in_=wdT[k * 128:(k + 1) * 128, :])
                for tt in range(NCHUNK):
                    for half in range(2):
                        nc.tensor.matmul(
                            dps[tt * 2 + half][:],
                            gu[:, k, tt * 128:(tt + 1) * 128],
                            wd_k[:, half * 512:(half + 1) * 512],
                            start=(k == 0), stop=(k == NFT - 1))
            for tt in range(NCHUNK):
                for half in range(2):
                    ob = pO.tile([128, 512], F32, tag="ob")
                    nc.vector.tensor_add(
                        ob[:], dps[tt * 2 + half][:],
                        h2[:, tt, half * 512:(half + 1) * 512])
                    tq[(tt * 2 + half) % 2].dma_start(
                        out=out_d[tt * 128:(tt + 1) * 128,
                                  half * 512:(half + 1) * 512],
                        in_=ob[:])
        es_g.close()
        es_go.close()
        es_yt.close()
        es_sz.close()

    nc.finalize()
    return nc


_CACHE = {}


def _get_program():
    if "p" not in _CACHE:
        _CACHE["p"] = build_program(None)
    return _CACHE["p"]


def kernel(hidden_states, w_ln1, w_in, w_conv, b_conv, dt_bias, A_log, D,
           w_mnorm, w_out, w_ln2, w_gate, w_up, w_down):
    bf = ml_dtypes.bfloat16
    hs = np.asarray(hidden_states, np.float32)
    wiTn = (np.asarray(w_in, np.float32) *
            np.asarray(w_ln1, np.float32)[None, :]).T.astype(bf)
    # column order: [dt pad128, xBC, z] then group tiling [9*128, 8*512]
    wi_pad = np.zeros((H, NGRP * 512), bf)
    wi_pad[:, 0:NH] = wiTn[:, E + CONV:D_IN]
    wi_pad[:, 128:128 + CONV] = wiTn[:, E:E + CONV]
    wi_pad[:, 128 + CONV:128 + CONV + E] = wiTn[:, 0:E]
    wiTn = wi_pad.reshape(NKH, 128, NGRP, 512).transpose(2, 1, 0, 3) \
        .reshape(NGRP * 128, NKH * 512)
    woTn = (np.asarray(w_out, np.float32) *
            np.asarray(w_mnorm, np.float32)[None, :]).T.astype(bf)
    wgTn = (np.asarray(w_gate, np.float32) *
            np.asarray(w_ln2, np.float32)[None, :]).T.astype(bf)
    wuTn = (np.asarray(w_up, np.float32) *
            np.asarray(w_ln2, np.float32)[None, :]).T.astype(bf)
    wgTn = wgTn.reshape(NKH, 128, NFT, 128).transpose(2, 1, 0, 3) \
        .reshape(NFT * 128, NKH * 128)
    wuTn = wuTn.reshape(NKH, 128, NFT, 128).transpose(2, 1, 0, 3) \
        .reshape(NFT * 128, NKH * 128)
    wdTn = np.asarray(w_down, np.float32).T.astype(bf)
    # conv taps as diagonal stationary blocks [128, (j,k)*128]
    wc = np.asarray(w_conv, np.float32).reshape(NXT, 128, KC)
    wdiag = np.zeros((128, NXT * KC * 128), bf)
    for j in range(NXT):
        for k in range(KC):
            blk = (j * KC + k) * 128
            wdiag[np.arange(128), blk + np.arange(128)] = wc[j, :, k].astype(bf)
    bconv = np.asarray(b_conv, np.float32).reshape(NXT, 128).T.copy()
    avec = (-np.exp(np.asarray(A_log, np.float32))).reshape(NH, 1)
    dtb = np.asarray(dt_bias, np.float32).reshape(NH, 1)
    negmask = (np.arange(128)[None, :] >= np.arange(128)[:, None]) \
        .astype(np.float32)
    # ddiag: diag(D per feature) blocks [128, j*128]
    Dv = np.asarray(D, np.float32)
    ddiag = np.zeros((128, NKE * 128), bf)
    for j in range(NKE):
        dv = np.empty(128, np.float32)
        dv[0:64] = Dv[2 * j]
        dv[64:128] = Dv[2 * j + 1]
        ddiag[np.arange(128), j * 128 + np.arange(128)] = dv.astype(bf)
    idf = np.eye(128, dtype=np.float32)

    nc = _get_program()

    shared = dict(wiT=np.ascontiguousarray(wiTn),
                  woT=np.ascontiguousarray(woTn),
                  wgT=np.ascontiguousarray(wgTn),
                  wuT=np.ascontiguousarray(wuTn),
                  wdT=np.ascontiguousarray(wdTn),
                  wdiag=np.ascontiguousarray(wdiag),
                  bconv=bconv, avec=avec, dtb=dtb,
                  negmask=negmask, idf32=idf, ddiag=np.ascontiguousarray(ddiag))
    in_maps = []
    for core in range(NCORES):
        b, r = core // 4, core % 4
        s0 = r * LSEQ
        hpad = np.zeros((NROW * 128, H), np.float32)
        hpad[HALO:HALO + LSEQ] = hs[b, s0:s0 + LSEQ]
        if s0 > 0:
            hpad[0:HALO] = hs[b, s0 - HALO:s0]
        m8 = np.zeros((128, 8), np.float32)
        for j in range(4):
            m8[:, j] = 1.0 if j < r else 0.0
            m8[:, 4 + j] = 0.0 if j < r else 1.0
        in_maps.append(dict(shared, hs=hpad, mask8=m8))

    res = run_bass_kernel_spmd(nc, in_maps, list(range(NCORES)))
    out = np.empty((2, 2048, H), np.float32)
    for core in range(NCORES):
        b, r = core // 4, core % 4
        out[b, r * LSEQ:(r + 1) * LSEQ] = res.results[core]["out"]
    return out
